# revision 14
# baseline (speedup 1.0000x reference)
"""MultiHeadSSM Trainium2 kernel (8 NeuronCores).

Module: xp = x @ W_in.T; dt = softplus(xp @ W_dt.T + b_dt);
a = exp(dt[...,None] * -exp(log_A)); linear scan s_t = a_t s_{t-1} + xp_t;
y = ys @ W_out.T; returns (y, final_state).

Sharding: 8 shards = batch(4) x T-halves(2). Core c handles b=c//2,
half=c&1 (2048 tokens, all 2048 channels). The cross-half scan dependency
is broken with the pair-scan decomposition: each core computes the local
zero-init scan U and the decay cumprod P = exp(A * cumsum(dt)); an 8KB
pair AllGather ships the first half's final state s_in, and
ys = U + P * s_in (s_in masked to 0 on even cores). dt is computed
directly from x via host-precomputed W_dteff = W_dt @ W_in, so the whole
pipeline is uniform SPMD with a single tiny collective.

On-device layout: channels on partitions, time on the free dim. Matmul
operands are bf16 (fp32 accumulation); the recurrence and all decay math
stay fp32. The recurrence runs on the DVE tensor_tensor_scan instruction.
Head->channel broadcasts are 0-stride DMA reads from DRAM, folded with
the per-channel A scale into the ACT exp.
"""
import sys

sys.path.insert(0, "/opt/trn_rl_repo")

import numpy as np

D = 2048          # d_model
H = 32            # heads
HD = 64           # head_dim
B = 4
T = 4096
NCORES = 8
TC = T // 2       # tokens per core
KB = D // 128     # 16 channel blocks
TT_A = 512        # phase-A time tile
NTA = TC // TT_A  # 4
TT_B = 512        # phase-B time tile
NTB = TC // TT_B  # 8
EW = 512          # phase-B out_proj e-chunk
NE = D // EW      # 4

_cache = {}


def _build():
    import concourse.bass as bass
    import concourse.bacc as bacc
    import concourse.tile as tile
    from concourse import mybir

    f32 = mybir.dt.float32
    bf16 = mybir.dt.bfloat16
    AF = mybir.ActivationFunctionType
    OP = mybir.AluOpType
    ts = bass.ts

    nc = bacc.Bacc("TRN2", target_bir_lowering=False, debug=False, num_devices=NCORES, num_swdge_queues=4)

    xt_d = nc.dram_tensor("xt", (128, NTA, KB, TT_A), bf16, kind="ExternalInput").ap()
    w_in_d = nc.dram_tensor("w_in_s", (KB, 128, KB, 128), bf16, kind="ExternalInput").ap()
    w_out_d = nc.dram_tensor("w_out_s", (128, KB, D), bf16, kind="ExternalInput").ap()
    w_dteff_d = nc.dram_tensor("w_dteff_s", (128, KB, H), bf16, kind="ExternalInput").ap()
    f32r = mybir.dt.float32r
    sa_d = nc.dram_tensor("sa", (H, D), f32r, kind="ExternalInput").ap()
    bdt_d = nc.dram_tensor("b_dt", (H, 1), f32, kind="ExternalInput").ap()
    cmask_d = nc.dram_tensor("cmask", (128, 1), f32, kind="ExternalInput").ap()
    y_d = nc.dram_tensor("y_out", (TC, D), f32, kind="ExternalOutput").ap()
    s_d = nc.dram_tensor("s_out", (128, KB), f32, kind="ExternalOutput").ap()

    with tile.TileContext(nc) as tc:
        with (
            tc.tile_pool(name="resident", bufs=1) as rpool,
            tc.tile_pool(name="cpool", bufs=1) as cpool,
            tc.tile_pool(name="dram", bufs=1, space="DRAM") as dram,
        ):
            u_dram = dram.tile([D, TC], f32)
            p_dram = dram.tile([D, TC], f32)
            cc_in1 = dram.tile([12, 128], f32)
            cc_out1 = dram.tile([2, 12, 128], f32)
            cc_in2 = dram.tile([4, 128], f32)
            cc_out2 = dram.tile([2, 4, 128], f32)
            u_r = u_dram[:].rearrange("(j p) t -> p j t", p=128)
            p_r = p_dram[:].rearrange("(j p) t -> p j t", p=128)

            # resident big tensors: x (bf16) and W_out (bf16) together
            xsb = rpool.tile([128, NTA, KB, TT_A], bf16)
            wout = rpool.tile([128, KB, D], bf16)
            for tci in range(NTA):
                nc.sync.dma_start(xsb[:, tci], xt_d[:, tci])
            nc.sync.dma_start(wout[:], w_out_d[:])

            wdteff = cpool.tile([128, KB, H], bf16)
            sa_sb = cpool.tile([H, D], f32r)
            dt_f32r = cpool.tile([H, TC], f32r)
            bdt_sb = cpool.tile([H, 1], f32)
            cmask_sb = cpool.tile([128, 1], f32)
            carry_u = cpool.tile([128, KB], f32)
            sfin_raw = cpool.tile([128, KB], f32)
            sfin = cpool.tile([128, KB], f32)
            s_stage = cpool.tile([128, KB], f32)
            carry_p = cpool.tile([128, KB], f32)
            zeros32 = cpool.tile([H, TT_A], f32)
            zeros128 = cpool.tile([128, TT_A], f32)
            carry_s32 = cpool.tile([H, 1], f32)
            nc.gpsimd.memset(zeros32[:], 0.0)
            nc.gpsimd.memset(zeros128[:], 0.0)
            nc.sync.dma_start(wdteff[:], w_dteff_d[:])
            nc.sync.dma_start(sa_sb[:], sa_d[:])
            nc.sync.dma_start(bdt_sb[:], bdt_d[:])
            nc.sync.dma_start(cmask_sb[:], cmask_d[:])

            # ---------------- phase A: dt, cumsum(dt), in_proj, scan U ------
            with (
                tc.tile_pool(name="psdt", bufs=2, space=bass.MemorySpace.PSUM) as psdt_pool,
                tc.tile_pool(name="psxp", bufs=4, space=bass.MemorySpace.PSUM) as psxp_pool,
                tc.tile_pool(name="psdtb", bufs=2, space=bass.MemorySpace.PSUM) as psdtb_pool,
                tc.tile_pool(name="win", bufs=3) as win_pool,
                tc.tile_pool(name="apool", bufs=4) as a_pool,
                tc.tile_pool(name="upool", bufs=4) as u_pool,
                tc.tile_pool(name="pspool", bufs=4) as pp_pool,
                tc.tile_pool(name="spool", bufs=2) as small_pool,
            ):
                wj_tiles = {}
                for j in range(2):
                    wj_tiles[j] = win_pool.tile([128, KB, 128], bf16, tag="wj", name=f"wj{j}")
                    nc.scalar.dma_start(wj_tiles[j][:], w_in_d[j])

                # dt = softplus(W_dteff @ x + b)
                for tci in range(NTA):
                    ps_dt = psdt_pool.tile([H, TT_A], f32)
                    for k in range(KB):
                        nc.tensor.matmul(
                            ps_dt[:],
                            wdteff[:, k, :],
                            xsb[:, tci, k, :],
                            start=(k == 0),
                            stop=(k == KB - 1),
                        )
                    e_sb = small_pool.tile([H, TT_A], f32, tag="esb")
                    nc.scalar.activation(e_sb[:], ps_dt[:], AF.Exp, bias=bdt_sb[:, 0:1])
                    nc.scalar.activation(
                        dt_f32r[:, ts(tci, TT_A)], e_sb[:], AF.Ln, bias=1.0
                    )

                for j in range(KB):
                    wj = wj_tiles.pop(j)
                    if j + 2 < KB:
                        wj_tiles[j + 2] = win_pool.tile([128, KB, 128], bf16, tag="wj", name=f"wj{j+2}")
                        nc.scalar.dma_start(wj_tiles[j + 2][:], w_in_d[j + 2])
                    for tci in range(NTA):
                        ps_xp = psxp_pool.tile([128, TT_A], f32)
                        for k in range(KB):
                            nc.tensor.matmul(
                                ps_xp[:],
                                wj[:, k, :],
                                xsb[:, tci, k, :],
                                start=(k == 0),
                                stop=(k == KB - 1),
                            )
                        ps_dtb = psdtb_pool.tile([128, TT_A], f32)
                        nc.tensor.matmul(
                            ps_dtb[:],
                            sa_sb[:, ts(j, 128)],
                            dt_f32r[:, ts(tci, TT_A)],
                            start=True,
                            stop=True,
                        )
                        a_sb = a_pool.tile([128, TT_A], f32)
                        nc.scalar.activation(a_sb[:], ps_dtb[:], AF.Exp)
                        u_sb = u_pool.tile([128, TT_A], f32)
                        init = 0.0 if tci == 0 else carry_u[:, j : j + 1]
                        nc.vector.tensor_tensor_scan(
                            u_sb[:], a_sb[:], ps_xp[:], init,
                            op0=OP.mult, op1=OP.add,
                        )
                        nc.vector.tensor_copy(
                            carry_u[:, j : j + 1], u_sb[:, TT_A - 1 : TT_A]
                        )
                        nc.sync.dma_start(u_r[:, j, ts(tci, TT_A)], u_sb[:])
                        p_sb = pp_pool.tile([128, TT_A], f32)
                        initp = 1.0 if tci == 0 else carry_p[:, j : j + 1]
                        nc.vector.tensor_tensor_scan(
                            p_sb[:], a_sb[:], zeros128[:], initp,
                            op0=OP.mult, op1=OP.add,
                        )
                        nc.vector.tensor_copy(
                            carry_p[:, j : j + 1], p_sb[:, TT_A - 1 : TT_A]
                        )
                        nc.sync.dma_start(p_r[:, j, ts(tci, TT_A)], p_sb[:])
                    if j == 11:
                        nc.sync.dma_start(
                            cc_in1[:].rearrange("j p -> p j"), carry_u[:, 0:12]
                        )
                        nc.gpsimd.collective_compute(
                            "AllGather",
                            OP.bypass,
                            replica_groups=[[0, 1], [2, 3], [4, 5], [6, 7]],
                            ins=[cc_in1.opt()],
                            outs=[cc_out1.opt()],
                        )

            # ---------------- pair handoff of final local states ------------
            nc.sync.dma_start(cc_in2[:].rearrange("j p -> p j"), carry_u[:, 12:16])
            nc.gpsimd.collective_compute(
                "AllGather",
                OP.bypass,
                replica_groups=[[0, 1], [2, 3], [4, 5], [6, 7]],
                ins=[cc_in2.opt()],
                outs=[cc_out2.opt()],
            )
            nc.sync.dma_start(sfin_raw[:, 0:12], cc_out1[0].rearrange("j p -> p j"))
            nc.sync.dma_start(sfin_raw[:, 12:16], cc_out2[0].rearrange("j p -> p j"))
            nc.vector.tensor_scalar_mul(
                sfin[:, 0:12], sfin_raw[:, 0:12], cmask_sb[:, 0:1]
            )
            nc.vector.tensor_scalar_mul(
                sfin[:, 12:16], sfin_raw[:, 12:16], cmask_sb[:, 0:1]
            )

            # ---------------- phase B: P, correction, out_proj --------------
            with (
                tc.tile_pool(name="psy", bufs=3, space=bass.MemorySpace.PSUM) as psy_pool,
                tc.tile_pool(name="uld", bufs=4) as uld_pool,
                tc.tile_pool(name="pld", bufs=4) as pld_pool,
                tc.tile_pool(name="yspool", bufs=2) as ys_pool,
                tc.tile_pool(name="ystage", bufs=2) as ystage_pool,
            ):
                for tci in range(NTB):
                    ys = ys_pool.tile([128, KB, TT_B], bf16, tag="ys")
                    for j in range(KB):
                        u_ld = uld_pool.tile([128, TT_B], f32)
                        nc.gpsimd.dma_start(u_ld[:], u_r[:, j, ts(tci, TT_B)])
                        p_sb = pld_pool.tile([128, TT_B], f32)
                        nc.gpsimd.dma_start(p_sb[:], p_r[:, j, ts(tci, TT_B)])
                        nc.vector.scalar_tensor_tensor(
                            ys[:, j, :], p_sb[:], sfin[:, j : j + 1], u_ld[:],
                            op0=OP.mult, op1=OP.add,
                        )
                        if tci == NTB - 1:
                            # final state in fp32: s = U_final + P_final * s_in
                            nc.vector.scalar_tensor_tensor(
                                s_stage[:, j : j + 1],
                                p_sb[:, TT_B - 1 : TT_B],
                                sfin[:, j : j + 1],
                                carry_u[:, j : j + 1],
                                op0=OP.mult, op1=OP.add,
                            )
                    for m in range(TT_B // 128):
                        for ne in range(NE):
                            ps_y = psy_pool.tile([128, EW], f32)
                            for j in range(KB):
                                nc.tensor.matmul(
                                    ps_y[:],
                                    ys[:, j, m * 128 : (m + 1) * 128],
                                    wout[:, j, ts(ne, EW)],
                                    start=(j == 0),
                                    stop=(j == KB - 1),
                                )
                            y_st = ystage_pool.tile([128, EW], f32)
                            nc.scalar.copy(y_st[:], ps_y[:])
                            row0 = tci * TT_B + m * 128
                            nc.sync.dma_start(
                                y_d[row0 : row0 + 128, ts(ne, EW)], y_st[:]
                            )
                nc.sync.dma_start(s_d[:], s_stage[:])

    nc.compile()
    return nc


def _get_nc():
    if "nc" not in _cache:
        _cache["nc"] = _build()
    return _cache["nc"]


def kernel(x, W_in, W_out, log_A, W_dt, b_dt):
    import ml_dtypes
    from concourse.bass_utils import run_bass_kernel_spmd

    bf16 = ml_dtypes.bfloat16
    x = np.asarray(x, dtype=np.float32)
    W_in = np.asarray(W_in, dtype=np.float32)
    W_out = np.asarray(W_out, dtype=np.float32)
    log_A = np.asarray(log_A, dtype=np.float32)
    W_dt = np.asarray(W_dt, dtype=np.float32)
    b_dt = np.asarray(b_dt, dtype=np.float32)

    nc = _get_nc()

    # (j, p, k, e): W_in[j*128+e, k*128+p]
    w_in_s = np.ascontiguousarray(
        W_in.T.reshape(KB, 128, KB, 128).transpose(2, 1, 0, 3)
    ).astype(bf16)
    # (p, k, e): W_out[e, k*128+p]
    w_out_s = np.ascontiguousarray(
        W_out.T.reshape(KB, 128, D).transpose(1, 0, 2)
    ).astype(bf16)
    W_dteff = (W_dt.astype(np.float64) @ W_in.astype(np.float64)).astype(np.float32)
    w_dteff_s = np.ascontiguousarray(
        W_dteff.T.reshape(KB, 128, H).transpose(1, 0, 2)
    ).astype(bf16)
    A_flat = (-np.exp(log_A.astype(np.float64))).astype(np.float32).reshape(D)
    SA = np.zeros((H, D), dtype=np.float32)
    SA[np.arange(D) // HD, np.arange(D)] = A_flat
    bdt_col = np.ascontiguousarray(b_dt.reshape(H, 1))

    in_maps = []
    for c in range(NCORES):
        b, half = c >> 1, c & 1
        xs = x[b, half * TC : (half + 1) * TC, :]  # (TC, D)
        # (p, tc, k, tt): x[b, .. tc*TT_A+tt, k*128+p]
        xt = np.ascontiguousarray(
            xs.reshape(NTA, TT_A, KB, 128).transpose(3, 0, 2, 1)
        ).astype(bf16)
        in_maps.append(
            {
                "xt": xt,
                "w_in_s": w_in_s,
                "w_out_s": w_out_s,
                "w_dteff_s": w_dteff_s,
                "sa": SA,
                "b_dt": bdt_col,
                "cmask": np.full((128, 1), float(half), dtype=np.float32),
            }
        )

    res = run_bass_kernel_spmd(nc, in_maps, core_ids=list(range(NCORES)))
    _cache["last_res"] = res

    y = np.empty((B, T, D), dtype=np.float32)
    s = np.empty((B, H, HD), dtype=np.float32)
    for c in range(NCORES):
        r = res.results[c]
        b, half = c >> 1, c & 1
        y[b, half * TC : (half + 1) * TC, :] = r["y_out"]
        if half == 1:
            s[b] = r["s_out"].T.reshape(H, HD)
    return y, s


# revision 15
# speedup vs baseline: 1.0486x; 1.0486x over previous
"""MultiHeadSSM Trainium2 kernel (8 NeuronCores).

Module: xp = x @ W_in.T; dt = softplus(xp @ W_dt.T + b_dt);
a = exp(dt[...,None] * -exp(log_A)); linear scan s_t = a_t s_{t-1} + xp_t;
y = ys @ W_out.T; returns (y, final_state).

Sharding: 8 shards = batch(4) x T-halves(2). Core c handles b=c//2,
half=c&1 (2048 tokens, all 2048 channels). The cross-half scan dependency
is broken with the pair-scan decomposition: each core computes the local
zero-init scan U and the decay cumprod P = exp(A * cumsum(dt)); an 8KB
pair AllGather ships the first half's final state s_in, and
ys = U + P * s_in (s_in masked to 0 on even cores). dt is computed
directly from x via host-precomputed W_dteff = W_dt @ W_in, so the whole
pipeline is uniform SPMD with a single tiny collective.

On-device layout: channels on partitions, time on the free dim. Matmul
operands are bf16 (fp32 accumulation); the recurrence and all decay math
stay fp32. The recurrence runs on the DVE tensor_tensor_scan instruction.
Head->channel broadcasts are 0-stride DMA reads from DRAM, folded with
the per-channel A scale into the ACT exp.
"""
import sys

sys.path.insert(0, "/opt/trn_rl_repo")

import numpy as np

D = 2048          # d_model
H = 32            # heads
HD = 64           # head_dim
B = 4
T = 4096
NCORES = 8
TC = T // 2       # tokens per core
KB = D // 128     # 16 channel blocks
TT_A = 512        # phase-A time tile
NTA = TC // TT_A  # 4
TT_B = 512        # phase-B time tile
NTB = TC // TT_B  # 8
EW = 512          # phase-B out_proj e-chunk
NE = D // EW      # 4

_cache = {}


def _build():
    import concourse.bass as bass
    import concourse.bacc as bacc
    import concourse.tile as tile
    from concourse import mybir

    f32 = mybir.dt.float32
    bf16 = mybir.dt.bfloat16
    AF = mybir.ActivationFunctionType
    OP = mybir.AluOpType
    ts = bass.ts

    nc = bacc.Bacc("TRN2", target_bir_lowering=False, debug=False, num_devices=NCORES, num_swdge_queues=4)

    xt_d = nc.dram_tensor("xt", (128, NTA, KB, TT_A), bf16, kind="ExternalInput").ap()
    w_in_d = nc.dram_tensor("w_in_s", (KB, 128, KB, 128), bf16, kind="ExternalInput").ap()
    w_out_d = nc.dram_tensor("w_out_s", (128, KB, D), bf16, kind="ExternalInput").ap()
    w_dteff_d = nc.dram_tensor("w_dteff_s", (128, KB, H), bf16, kind="ExternalInput").ap()
    f32r = mybir.dt.float32r
    sa_d = nc.dram_tensor("sa", (H, D), f32r, kind="ExternalInput").ap()
    bdt_d = nc.dram_tensor("b_dt", (H, 1), f32, kind="ExternalInput").ap()
    cmask_d = nc.dram_tensor("cmask", (128, 1), f32, kind="ExternalInput").ap()
    y_d = nc.dram_tensor("y_out", (TC, D), f32, kind="ExternalOutput").ap()
    s_d = nc.dram_tensor("s_out", (128, KB), f32, kind="ExternalOutput").ap()

    with tile.TileContext(nc) as tc:
        with (
            tc.tile_pool(name="resident", bufs=1) as rpool,
            tc.tile_pool(name="cpool", bufs=1) as cpool,
            tc.tile_pool(name="dram", bufs=1, space="DRAM") as dram,
        ):
            u_dram = dram.tile([D, TC], f32)
            p_dram = dram.tile([D, TC], f32)
            cc_in1 = dram.tile([12, 128], f32)
            cc_out1 = dram.tile([2, 12, 128], f32)
            cc_in2 = dram.tile([4, 128], f32)
            cc_out2 = dram.tile([2, 4, 128], f32)
            u_r = u_dram[:].rearrange("(j p) t -> p j t", p=128)
            p_r = p_dram[:].rearrange("(j p) t -> p j t", p=128)

            # resident big tensors: x (bf16) and W_out (bf16) together
            xsb = rpool.tile([128, NTA, KB, TT_A], bf16)
            wout = rpool.tile([128, KB, D], bf16)
            for tci in range(NTA):
                nc.sync.dma_start(xsb[:, tci], xt_d[:, tci])

            wdteff = cpool.tile([128, KB, H], bf16)
            sa_sb = cpool.tile([H, D], f32r)
            dt_f32r = cpool.tile([H, TC], f32r)
            bdt_sb = cpool.tile([H, 1], f32)
            cmask_sb = cpool.tile([128, 1], f32)
            carry_u = cpool.tile([128, KB], f32)
            sfin_raw = cpool.tile([128, KB], f32)
            sfin = cpool.tile([128, KB], f32)
            s_stage = cpool.tile([128, KB], f32)
            carry_p = cpool.tile([128, KB], f32)
            zeros32 = cpool.tile([H, TT_A], f32)
            zeros128 = cpool.tile([128, TT_A], f32)
            carry_s32 = cpool.tile([H, 1], f32)
            nc.gpsimd.memset(zeros32[:], 0.0)
            nc.gpsimd.memset(zeros128[:], 0.0)
            nc.sync.dma_start(wdteff[:], w_dteff_d[:])
            nc.sync.dma_start(sa_sb[:], sa_d[:])
            nc.sync.dma_start(bdt_sb[:], bdt_d[:])
            nc.sync.dma_start(cmask_sb[:], cmask_d[:])

            # ---------------- phase A: dt, cumsum(dt), in_proj, scan U ------
            with (
                tc.tile_pool(name="psdt", bufs=2, space=bass.MemorySpace.PSUM) as psdt_pool,
                tc.tile_pool(name="psxp", bufs=4, space=bass.MemorySpace.PSUM) as psxp_pool,
                tc.tile_pool(name="psdtb", bufs=2, space=bass.MemorySpace.PSUM) as psdtb_pool,
                tc.tile_pool(name="win", bufs=3) as win_pool,
                tc.tile_pool(name="apool", bufs=4) as a_pool,
                tc.tile_pool(name="upool", bufs=4) as u_pool,
                tc.tile_pool(name="pspool", bufs=4) as pp_pool,
                tc.tile_pool(name="spool", bufs=2) as small_pool,
            ):
                wj_tiles = {}
                for j in range(2):
                    wj_tiles[j] = win_pool.tile([128, KB, 128], bf16, tag="wj", name=f"wj{j}")
                    nc.gpsimd.dma_start(wj_tiles[j][:], w_in_d[j])

                # dt = softplus(W_dteff @ x + b)
                for tci in range(NTA):
                    ps_dt = psdt_pool.tile([H, TT_A], f32)
                    for k in range(KB):
                        nc.tensor.matmul(
                            ps_dt[:],
                            wdteff[:, k, :],
                            xsb[:, tci, k, :],
                            start=(k == 0),
                            stop=(k == KB - 1),
                        )
                    e_sb = small_pool.tile([H, TT_A], f32, tag="esb")
                    nc.scalar.activation(e_sb[:], ps_dt[:], AF.Exp, bias=bdt_sb[:, 0:1])
                    nc.scalar.activation(
                        dt_f32r[:, ts(tci, TT_A)], e_sb[:], AF.Ln, bias=1.0
                    )

                for k in range(KB):
                    nc.sync.dma_start(wout[:, k, :], w_out_d[:, k, :])

                for j in range(KB):
                    wj = wj_tiles.pop(j)
                    if j + 2 < KB:
                        wj_tiles[j + 2] = win_pool.tile([128, KB, 128], bf16, tag="wj", name=f"wj{j+2}")
                        nc.gpsimd.dma_start(wj_tiles[j + 2][:], w_in_d[j + 2])
                    for tci in range(NTA):
                        ps_xp = psxp_pool.tile([128, TT_A], f32)
                        for k in range(KB):
                            nc.tensor.matmul(
                                ps_xp[:],
                                wj[:, k, :],
                                xsb[:, tci, k, :],
                                start=(k == 0),
                                stop=(k == KB - 1),
                            )
                        ps_dtb = psdtb_pool.tile([128, TT_A], f32)
                        nc.tensor.matmul(
                            ps_dtb[:],
                            sa_sb[:, ts(j, 128)],
                            dt_f32r[:, ts(tci, TT_A)],
                            start=True,
                            stop=True,
                        )
                        a_sb = a_pool.tile([128, TT_A], f32)
                        nc.scalar.activation(a_sb[:], ps_dtb[:], AF.Exp)
                        u_sb = u_pool.tile([128, TT_A], f32)
                        init = 0.0 if tci == 0 else carry_u[:, j : j + 1]
                        nc.vector.tensor_tensor_scan(
                            u_sb[:], a_sb[:], ps_xp[:], init,
                            op0=OP.mult, op1=OP.add,
                        )
                        nc.vector.tensor_copy(
                            carry_u[:, j : j + 1], u_sb[:, TT_A - 1 : TT_A]
                        )
                        nc.sync.dma_start(u_r[:, j, ts(tci, TT_A)], u_sb[:])
                        p_sb = pp_pool.tile([128, TT_A], f32)
                        initp = 1.0 if tci == 0 else carry_p[:, j : j + 1]
                        nc.vector.tensor_tensor_scan(
                            p_sb[:], a_sb[:], zeros128[:], initp,
                            op0=OP.mult, op1=OP.add,
                        )
                        nc.vector.tensor_copy(
                            carry_p[:, j : j + 1], p_sb[:, TT_A - 1 : TT_A]
                        )
                        nc.sync.dma_start(p_r[:, j, ts(tci, TT_A)], p_sb[:])
                    if j == 11:
                        nc.sync.dma_start(
                            cc_in1[:].rearrange("j p -> p j"), carry_u[:, 0:12]
                        )
                        nc.gpsimd.collective_compute(
                            "AllGather",
                            OP.bypass,
                            replica_groups=[[0, 1], [2, 3], [4, 5], [6, 7]],
                            ins=[cc_in1.opt()],
                            outs=[cc_out1.opt()],
                        )

            # ---------------- pair handoff of final local states ------------
            nc.sync.dma_start(cc_in2[:].rearrange("j p -> p j"), carry_u[:, 12:16])
            nc.gpsimd.collective_compute(
                "AllGather",
                OP.bypass,
                replica_groups=[[0, 1], [2, 3], [4, 5], [6, 7]],
                ins=[cc_in2.opt()],
                outs=[cc_out2.opt()],
            )
            nc.sync.dma_start(sfin_raw[:, 0:12], cc_out1[0].rearrange("j p -> p j"))
            nc.sync.dma_start(sfin_raw[:, 12:16], cc_out2[0].rearrange("j p -> p j"))
            nc.vector.tensor_scalar_mul(
                sfin[:, 0:12], sfin_raw[:, 0:12], cmask_sb[:, 0:1]
            )
            nc.vector.tensor_scalar_mul(
                sfin[:, 12:16], sfin_raw[:, 12:16], cmask_sb[:, 0:1]
            )

            # ---------------- phase B: P, correction, out_proj --------------
            with (
                tc.tile_pool(name="psy", bufs=3, space=bass.MemorySpace.PSUM) as psy_pool,
                tc.tile_pool(name="uld", bufs=4) as uld_pool,
                tc.tile_pool(name="pld", bufs=4) as pld_pool,
                tc.tile_pool(name="yspool", bufs=2) as ys_pool,
                tc.tile_pool(name="ystage", bufs=2) as ystage_pool,
            ):
                for tci in range(NTB):
                    ys = ys_pool.tile([128, KB, TT_B], bf16, tag="ys")
                    for j in range(KB):
                        u_ld = uld_pool.tile([128, TT_B], f32)
                        nc.sync.dma_start(u_ld[:], u_r[:, j, ts(tci, TT_B)])
                        p_sb = pld_pool.tile([128, TT_B], f32)
                        nc.gpsimd.dma_start(p_sb[:], p_r[:, j, ts(tci, TT_B)])
                        nc.vector.scalar_tensor_tensor(
                            ys[:, j, :], p_sb[:], sfin[:, j : j + 1], u_ld[:],
                            op0=OP.mult, op1=OP.add,
                        )
                        if tci == NTB - 1:
                            # final state in fp32: s = U_final + P_final * s_in
                            nc.vector.scalar_tensor_tensor(
                                s_stage[:, j : j + 1],
                                p_sb[:, TT_B - 1 : TT_B],
                                sfin[:, j : j + 1],
                                carry_u[:, j : j + 1],
                                op0=OP.mult, op1=OP.add,
                            )
                    for m in range(TT_B // 128):
                        for ne in range(NE):
                            ps_y = psy_pool.tile([128, EW], f32)
                            for j in range(KB):
                                nc.tensor.matmul(
                                    ps_y[:],
                                    ys[:, j, m * 128 : (m + 1) * 128],
                                    wout[:, j, ts(ne, EW)],
                                    start=(j == 0),
                                    stop=(j == KB - 1),
                                )
                            y_st = ystage_pool.tile([128, EW], f32)
                            nc.scalar.copy(y_st[:], ps_y[:])
                            row0 = tci * TT_B + m * 128
                            nc.sync.dma_start(
                                y_d[row0 : row0 + 128, ts(ne, EW)], y_st[:]
                            )
                nc.sync.dma_start(s_d[:], s_stage[:])

    nc.compile()
    return nc


def _get_nc():
    if "nc" not in _cache:
        _cache["nc"] = _build()
    return _cache["nc"]


def kernel(x, W_in, W_out, log_A, W_dt, b_dt):
    import ml_dtypes
    from concourse.bass_utils import run_bass_kernel_spmd

    bf16 = ml_dtypes.bfloat16
    x = np.asarray(x, dtype=np.float32)
    W_in = np.asarray(W_in, dtype=np.float32)
    W_out = np.asarray(W_out, dtype=np.float32)
    log_A = np.asarray(log_A, dtype=np.float32)
    W_dt = np.asarray(W_dt, dtype=np.float32)
    b_dt = np.asarray(b_dt, dtype=np.float32)

    nc = _get_nc()

    # (j, p, k, e): W_in[j*128+e, k*128+p]
    w_in_s = np.ascontiguousarray(
        W_in.T.reshape(KB, 128, KB, 128).transpose(2, 1, 0, 3)
    ).astype(bf16)
    # (p, k, e): W_out[e, k*128+p]
    w_out_s = np.ascontiguousarray(
        W_out.T.reshape(KB, 128, D).transpose(1, 0, 2)
    ).astype(bf16)
    W_dteff = (W_dt.astype(np.float64) @ W_in.astype(np.float64)).astype(np.float32)
    w_dteff_s = np.ascontiguousarray(
        W_dteff.T.reshape(KB, 128, H).transpose(1, 0, 2)
    ).astype(bf16)
    A_flat = (-np.exp(log_A.astype(np.float64))).astype(np.float32).reshape(D)
    SA = np.zeros((H, D), dtype=np.float32)
    SA[np.arange(D) // HD, np.arange(D)] = A_flat
    bdt_col = np.ascontiguousarray(b_dt.reshape(H, 1))

    in_maps = []
    for c in range(NCORES):
        b, half = c >> 1, c & 1
        xs = x[b, half * TC : (half + 1) * TC, :]  # (TC, D)
        # (p, tc, k, tt): x[b, .. tc*TT_A+tt, k*128+p]
        xt = np.ascontiguousarray(
            xs.reshape(NTA, TT_A, KB, 128).transpose(3, 0, 2, 1)
        ).astype(bf16)
        in_maps.append(
            {
                "xt": xt,
                "w_in_s": w_in_s,
                "w_out_s": w_out_s,
                "w_dteff_s": w_dteff_s,
                "sa": SA,
                "b_dt": bdt_col,
                "cmask": np.full((128, 1), float(half), dtype=np.float32),
            }
        )

    res = run_bass_kernel_spmd(nc, in_maps, core_ids=list(range(NCORES)))
    _cache["last_res"] = res

    y = np.empty((B, T, D), dtype=np.float32)
    s = np.empty((B, H, HD), dtype=np.float32)
    for c in range(NCORES):
        r = res.results[c]
        b, half = c >> 1, c & 1
        y[b, half * TC : (half + 1) * TC, :] = r["y_out"]
        if half == 1:
            s[b] = r["s_out"].T.reshape(H, HD)
    return y, s


# revision 16
# speedup vs baseline: 1.1222x; 1.0702x over previous
"""MultiHeadSSM Trainium2 kernel (8 NeuronCores).

Module: xp = x @ W_in.T; dt = softplus(xp @ W_dt.T + b_dt);
a = exp(dt[...,None] * -exp(log_A)); linear scan s_t = a_t s_{t-1} + xp_t;
y = ys @ W_out.T; returns (y, final_state).

Sharding: 8 shards = batch(4) x T-halves(2). Core c handles b=c//2,
half=c&1 (2048 tokens, all 2048 channels). The cross-half scan dependency
is broken with the pair-scan decomposition: each core computes the local
zero-init scan U and the decay cumprod P = exp(A * cumsum(dt)); an 8KB
pair AllGather ships the first half's final state s_in, and
ys = U + P * s_in (s_in masked to 0 on even cores). dt is computed
directly from x via host-precomputed W_dteff = W_dt @ W_in, so the whole
pipeline is uniform SPMD with a single tiny collective.

On-device layout: channels on partitions, time on the free dim. Matmul
operands are bf16 (fp32 accumulation); the recurrence and all decay math
stay fp32. The recurrence runs on the DVE tensor_tensor_scan instruction.
Head->channel broadcasts are 0-stride DMA reads from DRAM, folded with
the per-channel A scale into the ACT exp.
"""
import sys

sys.path.insert(0, "/opt/trn_rl_repo")

import numpy as np

D = 2048          # d_model
H = 32            # heads
HD = 64           # head_dim
B = 4
T = 4096
NCORES = 8
TC = T // 2       # tokens per core
KB = D // 128     # 16 channel blocks
TT_A = 512        # phase-A time tile
NTA = TC // TT_A  # 4
TT_B = 512        # phase-B time tile
NTB = TC // TT_B  # 8
EW = 512          # phase-B out_proj e-chunk
NE = D // EW      # 4

_cache = {}


def _build():
    import concourse.bass as bass
    import concourse.bacc as bacc
    import concourse.tile as tile
    from concourse import mybir

    f32 = mybir.dt.float32
    bf16 = mybir.dt.bfloat16
    AF = mybir.ActivationFunctionType
    OP = mybir.AluOpType
    ts = bass.ts

    nc = bacc.Bacc("TRN2", target_bir_lowering=False, debug=False, num_devices=NCORES, num_swdge_queues=4)

    xt_d = nc.dram_tensor("xt", (128, NTA, KB, TT_A), bf16, kind="ExternalInput").ap()
    w_in_d = nc.dram_tensor("w_in_s", (KB, 128, KB, 128), bf16, kind="ExternalInput").ap()
    w_out_d = nc.dram_tensor("w_out_s", (128, KB, D), bf16, kind="ExternalInput").ap()
    w_dteff_d = nc.dram_tensor("w_dteff_s", (128, KB, H), bf16, kind="ExternalInput").ap()
    f32r = mybir.dt.float32r
    sa_d = nc.dram_tensor("sa", (H, D), f32r, kind="ExternalInput").ap()
    bdt_d = nc.dram_tensor("b_dt", (H, 1), f32, kind="ExternalInput").ap()
    cmask_d = nc.dram_tensor("cmask", (128, 1), f32, kind="ExternalInput").ap()
    y_d = nc.dram_tensor("y_out", (TC, D), f32, kind="ExternalOutput").ap()
    s_d = nc.dram_tensor("s_out", (128, KB), f32, kind="ExternalOutput").ap()

    with tile.TileContext(nc) as tc:
        with (
            tc.tile_pool(name="resident", bufs=1) as rpool,
            tc.tile_pool(name="cpool", bufs=1) as cpool,
            tc.tile_pool(name="dram", bufs=1, space="DRAM") as dram,
        ):
            u_dram = dram.tile([D, TC], bf16)
            p_dram = dram.tile([D, TC], bf16)
            cc_in1 = dram.tile([12, 128], f32)
            cc_out1 = dram.tile([2, 12, 128], f32)
            cc_in2 = dram.tile([4, 128], f32)
            cc_out2 = dram.tile([2, 4, 128], f32)
            u_r = u_dram[:].rearrange("(j p) t -> p j t", p=128)
            p_r = p_dram[:].rearrange("(j p) t -> p j t", p=128)

            # resident big tensors: x (bf16) and W_out (bf16) together
            xsb = rpool.tile([128, NTA, KB, TT_A], bf16)
            wout = rpool.tile([128, KB, D], bf16)
            for tci in range(NTA):
                for kq in range(4):
                    nc.sync.dma_start(
                        xsb[:, tci, 4 * kq : 4 * kq + 4, :],
                        xt_d[:, tci, 4 * kq : 4 * kq + 4, :],
                    )

            wdteff = cpool.tile([128, KB, H], bf16)
            sa_sb = cpool.tile([H, D], f32r)
            dt_f32r = cpool.tile([H, TC], f32r)
            bdt_sb = cpool.tile([H, 1], f32)
            cmask_sb = cpool.tile([128, 1], f32)
            carry_u = cpool.tile([128, KB], f32)
            sfin_raw = cpool.tile([128, KB], f32)
            sfin = cpool.tile([128, KB], f32)
            s_stage = cpool.tile([128, KB], f32)
            carry_p = cpool.tile([128, KB], f32)
            zeros32 = cpool.tile([H, TT_A], f32)
            zeros128 = cpool.tile([128, TT_A], f32)
            carry_s32 = cpool.tile([H, 1], f32)
            nc.gpsimd.memset(zeros32[:], 0.0)
            nc.gpsimd.memset(zeros128[:], 0.0)
            nc.sync.dma_start(wdteff[:], w_dteff_d[:])
            nc.sync.dma_start(sa_sb[:], sa_d[:])
            nc.sync.dma_start(bdt_sb[:], bdt_d[:])
            nc.sync.dma_start(cmask_sb[:], cmask_d[:])

            # ---------------- phase A: dt, cumsum(dt), in_proj, scan U ------
            with (
                tc.tile_pool(name="psdt", bufs=2, space=bass.MemorySpace.PSUM) as psdt_pool,
                tc.tile_pool(name="psxp", bufs=4, space=bass.MemorySpace.PSUM) as psxp_pool,
                tc.tile_pool(name="psdtb", bufs=2, space=bass.MemorySpace.PSUM) as psdtb_pool,
                tc.tile_pool(name="win", bufs=3) as win_pool,
                tc.tile_pool(name="apool", bufs=4) as a_pool,
                tc.tile_pool(name="upool", bufs=4) as u_pool,
                tc.tile_pool(name="pspool", bufs=4) as pp_pool,
                tc.tile_pool(name="spool", bufs=2) as small_pool,
            ):
                wj_tiles = {}
                for j in range(2):
                    wj_tiles[j] = win_pool.tile([128, KB, 128], bf16, tag="wj", name=f"wj{j}")
                    nc.gpsimd.dma_start(wj_tiles[j][:], w_in_d[j])

                # dt = softplus(W_dteff @ x + b)
                for tci in range(NTA):
                    ps_dt = psdt_pool.tile([H, TT_A], f32)
                    for k in range(KB):
                        nc.tensor.matmul(
                            ps_dt[:],
                            wdteff[:, k, :],
                            xsb[:, tci, k, :],
                            start=(k == 0),
                            stop=(k == KB - 1),
                        )
                    e_sb = small_pool.tile([H, TT_A], f32, tag="esb")
                    nc.scalar.activation(e_sb[:], ps_dt[:], AF.Exp, bias=bdt_sb[:, 0:1])
                    nc.scalar.activation(
                        dt_f32r[:, ts(tci, TT_A)], e_sb[:], AF.Ln, bias=1.0
                    )

                for k in range(KB):
                    nc.sync.dma_start(wout[:, k, :], w_out_d[:, k, :])

                for j in range(KB):
                    wj = wj_tiles.pop(j)
                    if j + 2 < KB:
                        wj_tiles[j + 2] = win_pool.tile([128, KB, 128], bf16, tag="wj", name=f"wj{j+2}")
                        nc.gpsimd.dma_start(wj_tiles[j + 2][:], w_in_d[j + 2])
                    for tci in range(NTA):
                        ps_xp = psxp_pool.tile([128, TT_A], f32)
                        for k in range(KB):
                            nc.tensor.matmul(
                                ps_xp[:],
                                wj[:, k, :],
                                xsb[:, tci, k, :],
                                start=(k == 0),
                                stop=(k == KB - 1),
                            )
                        ps_dtb = psdtb_pool.tile([128, TT_A], f32)
                        nc.tensor.matmul(
                            ps_dtb[:],
                            sa_sb[:, ts(j, 128)],
                            dt_f32r[:, ts(tci, TT_A)],
                            start=True,
                            stop=True,
                        )
                        a_sb = a_pool.tile([128, TT_A], f32)
                        nc.scalar.activation(a_sb[:], ps_dtb[:], AF.Exp)
                        u_sb = u_pool.tile([128, TT_A], bf16)
                        init = 0.0 if tci == 0 else carry_u[:, j : j + 1]
                        nc.vector.tensor_tensor_scan(
                            u_sb[:], a_sb[:], ps_xp[:], init,
                            op0=OP.mult, op1=OP.add,
                        )
                        nc.vector.tensor_copy(
                            carry_u[:, j : j + 1], u_sb[:, TT_A - 1 : TT_A]
                        )
                        nc.sync.dma_start(u_r[:, j, ts(tci, TT_A)], u_sb[:])
                        p_sb = pp_pool.tile([128, TT_A], bf16)
                        initp = 1.0 if tci == 0 else carry_p[:, j : j + 1]
                        nc.vector.tensor_tensor_scan(
                            p_sb[:], a_sb[:], zeros128[:], initp,
                            op0=OP.mult, op1=OP.add,
                        )
                        nc.vector.tensor_copy(
                            carry_p[:, j : j + 1], p_sb[:, TT_A - 1 : TT_A]
                        )
                        nc.sync.dma_start(p_r[:, j, ts(tci, TT_A)], p_sb[:])
                    if j == 11:
                        nc.sync.dma_start(
                            cc_in1[:].rearrange("j p -> p j"), carry_u[:, 0:12]
                        )
                        nc.gpsimd.collective_compute(
                            "AllGather",
                            OP.bypass,
                            replica_groups=[[0, 1], [2, 3], [4, 5], [6, 7]],
                            ins=[cc_in1.opt()],
                            outs=[cc_out1.opt()],
                        )

            # ---------------- pair handoff of final local states ------------
            nc.sync.dma_start(cc_in2[:].rearrange("j p -> p j"), carry_u[:, 12:16])
            nc.gpsimd.collective_compute(
                "AllGather",
                OP.bypass,
                replica_groups=[[0, 1], [2, 3], [4, 5], [6, 7]],
                ins=[cc_in2.opt()],
                outs=[cc_out2.opt()],
            )
            nc.sync.dma_start(sfin_raw[:, 0:12], cc_out1[0].rearrange("j p -> p j"))
            nc.sync.dma_start(sfin_raw[:, 12:16], cc_out2[0].rearrange("j p -> p j"))
            nc.vector.tensor_scalar_mul(
                sfin[:, 0:12], sfin_raw[:, 0:12], cmask_sb[:, 0:1]
            )
            nc.vector.tensor_scalar_mul(
                sfin[:, 12:16], sfin_raw[:, 12:16], cmask_sb[:, 0:1]
            )

            # ---------------- phase B: P, correction, out_proj --------------
            with (
                tc.tile_pool(name="psy", bufs=3, space=bass.MemorySpace.PSUM) as psy_pool,
                tc.tile_pool(name="uld", bufs=4) as uld_pool,
                tc.tile_pool(name="pld", bufs=4) as pld_pool,
                tc.tile_pool(name="yspool", bufs=2) as ys_pool,
                tc.tile_pool(name="ystage", bufs=2) as ystage_pool,
            ):
                for tci in range(NTB):
                    ys = ys_pool.tile([128, KB, TT_B], bf16, tag="ys")
                    for j in range(KB):
                        u_ld = uld_pool.tile([128, TT_B], bf16)
                        nc.sync.dma_start(u_ld[:], u_r[:, j, ts(tci, TT_B)])
                        p_sb = pld_pool.tile([128, TT_B], bf16)
                        nc.gpsimd.dma_start(p_sb[:], p_r[:, j, ts(tci, TT_B)])
                        nc.vector.scalar_tensor_tensor(
                            ys[:, j, :], p_sb[:], sfin[:, j : j + 1], u_ld[:],
                            op0=OP.mult, op1=OP.add,
                        )
                        if tci == NTB - 1:
                            # final state in fp32: s = U_final + P_final * s_in
                            nc.vector.scalar_tensor_tensor(
                                s_stage[:, j : j + 1],
                                p_sb[:, TT_B - 1 : TT_B],
                                sfin[:, j : j + 1],
                                carry_u[:, j : j + 1],
                                op0=OP.mult, op1=OP.add,
                            )
                    for m in range(TT_B // 128):
                        for ne in range(NE):
                            ps_y = psy_pool.tile([128, EW], f32)
                            for j in range(KB):
                                nc.tensor.matmul(
                                    ps_y[:],
                                    ys[:, j, m * 128 : (m + 1) * 128],
                                    wout[:, j, ts(ne, EW)],
                                    start=(j == 0),
                                    stop=(j == KB - 1),
                                )
                            y_st = ystage_pool.tile([128, EW], f32)
                            nc.scalar.copy(y_st[:], ps_y[:])
                            row0 = tci * TT_B + m * 128
                            nc.sync.dma_start(
                                y_d[row0 : row0 + 128, ts(ne, EW)], y_st[:]
                            )
                nc.sync.dma_start(s_d[:], s_stage[:])

    nc.compile()
    return nc


def _get_nc():
    if "nc" not in _cache:
        _cache["nc"] = _build()
    return _cache["nc"]


def kernel(x, W_in, W_out, log_A, W_dt, b_dt):
    import ml_dtypes
    from concourse.bass_utils import run_bass_kernel_spmd

    bf16 = ml_dtypes.bfloat16
    x = np.asarray(x, dtype=np.float32)
    W_in = np.asarray(W_in, dtype=np.float32)
    W_out = np.asarray(W_out, dtype=np.float32)
    log_A = np.asarray(log_A, dtype=np.float32)
    W_dt = np.asarray(W_dt, dtype=np.float32)
    b_dt = np.asarray(b_dt, dtype=np.float32)

    nc = _get_nc()

    # (j, p, k, e): W_in[j*128+e, k*128+p]
    w_in_s = np.ascontiguousarray(
        W_in.T.reshape(KB, 128, KB, 128).transpose(2, 1, 0, 3)
    ).astype(bf16)
    # (p, k, e): W_out[e, k*128+p]
    w_out_s = np.ascontiguousarray(
        W_out.T.reshape(KB, 128, D).transpose(1, 0, 2)
    ).astype(bf16)
    W_dteff = (W_dt.astype(np.float64) @ W_in.astype(np.float64)).astype(np.float32)
    w_dteff_s = np.ascontiguousarray(
        W_dteff.T.reshape(KB, 128, H).transpose(1, 0, 2)
    ).astype(bf16)
    A_flat = (-np.exp(log_A.astype(np.float64))).astype(np.float32).reshape(D)
    SA = np.zeros((H, D), dtype=np.float32)
    SA[np.arange(D) // HD, np.arange(D)] = A_flat
    bdt_col = np.ascontiguousarray(b_dt.reshape(H, 1))

    in_maps = []
    for c in range(NCORES):
        b, half = c >> 1, c & 1
        xs = x[b, half * TC : (half + 1) * TC, :]  # (TC, D)
        # (p, tc, k, tt): x[b, .. tc*TT_A+tt, k*128+p]
        xt = np.ascontiguousarray(
            xs.reshape(NTA, TT_A, KB, 128).transpose(3, 0, 2, 1)
        ).astype(bf16)
        in_maps.append(
            {
                "xt": xt,
                "w_in_s": w_in_s,
                "w_out_s": w_out_s,
                "w_dteff_s": w_dteff_s,
                "sa": SA,
                "b_dt": bdt_col,
                "cmask": np.full((128, 1), float(half), dtype=np.float32),
            }
        )

    res = run_bass_kernel_spmd(nc, in_maps, core_ids=list(range(NCORES)))
    _cache["last_res"] = res

    y = np.empty((B, T, D), dtype=np.float32)
    s = np.empty((B, H, HD), dtype=np.float32)
    for c in range(NCORES):
        r = res.results[c]
        b, half = c >> 1, c & 1
        y[b, half * TC : (half + 1) * TC, :] = r["y_out"]
        if half == 1:
            s[b] = r["s_out"].T.reshape(H, HD)
    return y, s


# revision 17
# speedup vs baseline: 1.1520x; 1.0266x over previous
"""MultiHeadSSM Trainium2 kernel (8 NeuronCores).

Module: xp = x @ W_in.T; dt = softplus(xp @ W_dt.T + b_dt);
a = exp(dt[...,None] * -exp(log_A)); linear scan s_t = a_t s_{t-1} + xp_t;
y = ys @ W_out.T; returns (y, final_state).

Sharding: 8 shards = batch(4) x T-halves(2). Core c handles b=c//2,
half=c&1 (2048 tokens, all 2048 channels). The cross-half scan dependency
is broken with the pair-scan decomposition: each core computes the local
zero-init scan U and the decay cumprod P = exp(A * cumsum(dt)); an 8KB
pair AllGather ships the first half's final state s_in, and
ys = U + P * s_in (s_in masked to 0 on even cores). dt is computed
directly from x via host-precomputed W_dteff = W_dt @ W_in, so the whole
pipeline is uniform SPMD with a single tiny collective.

On-device layout: channels on partitions, time on the free dim. Matmul
operands are bf16 (fp32 accumulation); the recurrence and all decay math
stay fp32. The recurrence runs on the DVE tensor_tensor_scan instruction.
Head->channel broadcasts are 0-stride DMA reads from DRAM, folded with
the per-channel A scale into the ACT exp.
"""
import sys

sys.path.insert(0, "/opt/trn_rl_repo")

import numpy as np

D = 2048          # d_model
H = 32            # heads
HD = 64           # head_dim
B = 4
T = 4096
NCORES = 8
TC = T // 2       # tokens per core
KB = D // 128     # 16 channel blocks
TT_A = 512        # phase-A time tile
NTA = TC // TT_A  # 4
TT_B = 512        # phase-B time tile
NTB = TC // TT_B  # 8
EW = 512          # phase-B out_proj e-chunk
NE = D // EW      # 4

_cache = {}


def _build():
    import concourse.bass as bass
    import concourse.bacc as bacc
    import concourse.tile as tile
    from concourse import mybir

    f32 = mybir.dt.float32
    bf16 = mybir.dt.bfloat16
    AF = mybir.ActivationFunctionType
    OP = mybir.AluOpType
    ts = bass.ts

    nc = bacc.Bacc("TRN2", target_bir_lowering=False, debug=False, num_devices=NCORES, num_swdge_queues=4)

    xt_d = nc.dram_tensor("xt", (128, NTA, KB, TT_A), bf16, kind="ExternalInput").ap()
    w_in_d = nc.dram_tensor("w_in_s", (KB, 128, KB, 128), bf16, kind="ExternalInput").ap()
    w_out_d = nc.dram_tensor("w_out_s", (128, KB, D), bf16, kind="ExternalInput").ap()
    w_dteff_d = nc.dram_tensor("w_dteff_s", (128, KB, H), bf16, kind="ExternalInput").ap()
    f32r = mybir.dt.float32r
    sa_d = nc.dram_tensor("sa", (H, D), f32r, kind="ExternalInput").ap()
    bdt_d = nc.dram_tensor("b_dt", (H, 1), f32, kind="ExternalInput").ap()
    cmask_d = nc.dram_tensor("cmask", (128, 1), f32, kind="ExternalInput").ap()
    y_d = nc.dram_tensor("y_out", (TC, D), f32, kind="ExternalOutput").ap()
    s_d = nc.dram_tensor("s_out", (128, KB), f32, kind="ExternalOutput").ap()

    with tile.TileContext(nc) as tc:
        with (
            tc.tile_pool(name="resident", bufs=1) as rpool,
            tc.tile_pool(name="cpool", bufs=1) as cpool,
            tc.tile_pool(name="dram", bufs=1, space="DRAM") as dram,
        ):
            u_dram = dram.tile([D, TC], bf16)
            p_dram = dram.tile([D, TC], bf16)
            cc_in1 = dram.tile([12, 128], f32)
            cc_out1 = dram.tile([2, 12, 128], f32)
            cc_in2 = dram.tile([4, 128], f32)
            cc_out2 = dram.tile([2, 4, 128], f32)
            u_r = u_dram[:].rearrange("(j p) t -> p j t", p=128)
            p_r = p_dram[:].rearrange("(j p) t -> p j t", p=128)

            # resident big tensors: x (bf16) and W_out (bf16) together
            xsb = rpool.tile([128, NTA, KB, TT_A], bf16)
            wout = rpool.tile([128, KB, D], bf16)
            for tci in range(NTA):
                for kq in range(4):
                    nc.sync.dma_start(
                        xsb[:, tci, 4 * kq : 4 * kq + 4, :],
                        xt_d[:, tci, 4 * kq : 4 * kq + 4, :],
                    )

            wdteff = cpool.tile([128, KB, H], bf16)
            sa_sb = cpool.tile([H, D], f32r)
            dt_f32r = cpool.tile([H, TC], f32r)
            bdt_sb = cpool.tile([H, 1], f32)
            cmask_sb = cpool.tile([128, 1], f32)
            carry_u = cpool.tile([128, KB], f32)
            sfin_raw = cpool.tile([128, KB], f32)
            sfin = cpool.tile([128, KB], f32)
            s_stage = cpool.tile([128, KB], f32)
            carry_p = cpool.tile([128, KB], f32)
            zeros32 = cpool.tile([H, TT_A], f32)
            zeros128 = cpool.tile([128, TT_A], f32)
            carry_s32 = cpool.tile([H, 1], f32)
            nc.gpsimd.memset(zeros32[:], 0.0)
            nc.gpsimd.memset(zeros128[:], 0.0)
            nc.sync.dma_start(wdteff[:], w_dteff_d[:])
            nc.sync.dma_start(sa_sb[:], sa_d[:])
            nc.sync.dma_start(bdt_sb[:], bdt_d[:])
            nc.sync.dma_start(cmask_sb[:], cmask_d[:])

            # ---------------- phase A: dt, cumsum(dt), in_proj, scan U ------
            with (
                tc.tile_pool(name="psdt", bufs=1, space=bass.MemorySpace.PSUM) as psdt_pool,
                tc.tile_pool(name="psxp", bufs=5, space=bass.MemorySpace.PSUM) as psxp_pool,
                tc.tile_pool(name="psdtb", bufs=2, space=bass.MemorySpace.PSUM) as psdtb_pool,
                tc.tile_pool(name="win", bufs=3) as win_pool,
                tc.tile_pool(name="apool", bufs=5) as a_pool,
                tc.tile_pool(name="upool", bufs=4) as u_pool,
                tc.tile_pool(name="pspool", bufs=4) as pp_pool,
                tc.tile_pool(name="spool", bufs=2) as small_pool,
            ):
                wj_tiles = {}
                for j in range(2):
                    wj_tiles[j] = win_pool.tile([128, KB, 128], bf16, tag="wj", name=f"wj{j}")
                    nc.gpsimd.dma_start(wj_tiles[j][:], w_in_d[j])

                # dt = softplus(W_dteff @ x + b)
                for tci in range(NTA):
                    ps_dt = psdt_pool.tile([H, TT_A], f32)
                    for k in range(KB):
                        nc.tensor.matmul(
                            ps_dt[:],
                            wdteff[:, k, :],
                            xsb[:, tci, k, :],
                            start=(k == 0),
                            stop=(k == KB - 1),
                        )
                    e_sb = small_pool.tile([H, TT_A], f32, tag="esb")
                    nc.scalar.activation(e_sb[:], ps_dt[:], AF.Exp, bias=bdt_sb[:, 0:1])
                    nc.scalar.activation(
                        dt_f32r[:, ts(tci, TT_A)], e_sb[:], AF.Ln, bias=1.0
                    )

                for k in range(KB):
                    nc.sync.dma_start(wout[:, k, :], w_out_d[:, k, :])

                for j in range(KB):
                    wj = wj_tiles.pop(j)
                    if j + 2 < KB:
                        wj_tiles[j + 2] = win_pool.tile([128, KB, 128], bf16, tag="wj", name=f"wj{j+2}")
                        nc.gpsimd.dma_start(wj_tiles[j + 2][:], w_in_d[j + 2])
                    for tci in range(NTA):
                        ps_xp = psxp_pool.tile([128, TT_A], f32)
                        for k in range(KB):
                            nc.tensor.matmul(
                                ps_xp[:],
                                wj[:, k, :],
                                xsb[:, tci, k, :],
                                start=(k == 0),
                                stop=(k == KB - 1),
                            )
                        ps_dtb = psdtb_pool.tile([128, TT_A], f32)
                        nc.tensor.matmul(
                            ps_dtb[:],
                            sa_sb[:, ts(j, 128)],
                            dt_f32r[:, ts(tci, TT_A)],
                            start=True,
                            stop=True,
                        )
                        a_sb = a_pool.tile([128, TT_A], f32)
                        nc.scalar.activation(a_sb[:], ps_dtb[:], AF.Exp)
                        u_sb = u_pool.tile([128, TT_A], bf16)
                        init = 0.0 if tci == 0 else carry_u[:, j : j + 1]
                        nc.vector.tensor_tensor_scan(
                            u_sb[:], a_sb[:], ps_xp[:], init,
                            op0=OP.mult, op1=OP.add,
                        )
                        nc.vector.tensor_copy(
                            carry_u[:, j : j + 1], u_sb[:, TT_A - 1 : TT_A]
                        )
                        nc.sync.dma_start(u_r[:, j, ts(tci, TT_A)], u_sb[:])
                        p_sb = pp_pool.tile([128, TT_A], bf16)
                        initp = 1.0 if tci == 0 else carry_p[:, j : j + 1]
                        nc.vector.tensor_tensor_scan(
                            p_sb[:], a_sb[:], zeros128[:], initp,
                            op0=OP.mult, op1=OP.add,
                        )
                        nc.vector.tensor_copy(
                            carry_p[:, j : j + 1], p_sb[:, TT_A - 1 : TT_A]
                        )
                        nc.sync.dma_start(p_r[:, j, ts(tci, TT_A)], p_sb[:])
                    if j == 11:
                        nc.sync.dma_start(
                            cc_in1[:].rearrange("j p -> p j"), carry_u[:, 0:12]
                        )
                        nc.gpsimd.collective_compute(
                            "AllGather",
                            OP.bypass,
                            replica_groups=[[0, 1], [2, 3], [4, 5], [6, 7]],
                            ins=[cc_in1.opt()],
                            outs=[cc_out1.opt()],
                        )

            # ---------------- pair handoff of final local states ------------
            nc.sync.dma_start(cc_in2[:].rearrange("j p -> p j"), carry_u[:, 12:16])
            nc.gpsimd.collective_compute(
                "AllGather",
                OP.bypass,
                replica_groups=[[0, 1], [2, 3], [4, 5], [6, 7]],
                ins=[cc_in2.opt()],
                outs=[cc_out2.opt()],
            )
            nc.sync.dma_start(sfin_raw[:, 0:12], cc_out1[0].rearrange("j p -> p j"))
            nc.sync.dma_start(sfin_raw[:, 12:16], cc_out2[0].rearrange("j p -> p j"))
            nc.vector.tensor_scalar_mul(
                sfin[:, 0:12], sfin_raw[:, 0:12], cmask_sb[:, 0:1]
            )
            nc.vector.tensor_scalar_mul(
                sfin[:, 12:16], sfin_raw[:, 12:16], cmask_sb[:, 0:1]
            )

            # ---------------- phase B: P, correction, out_proj --------------
            with (
                tc.tile_pool(name="psy", bufs=3, space=bass.MemorySpace.PSUM) as psy_pool,
                tc.tile_pool(name="uld", bufs=6) as uld_pool,
                tc.tile_pool(name="pld", bufs=6) as pld_pool,
                tc.tile_pool(name="yspool", bufs=2) as ys_pool,
                tc.tile_pool(name="ystage", bufs=2) as ystage_pool,
            ):
                for tci in range(NTB):
                    ys = ys_pool.tile([128, KB, TT_B], bf16, tag="ys")
                    for j in range(KB):
                        u_ld = uld_pool.tile([128, TT_B], bf16)
                        nc.sync.dma_start(u_ld[:], u_r[:, j, ts(tci, TT_B)])
                        p_sb = pld_pool.tile([128, TT_B], bf16)
                        nc.gpsimd.dma_start(p_sb[:], p_r[:, j, ts(tci, TT_B)])
                        nc.vector.scalar_tensor_tensor(
                            ys[:, j, :], p_sb[:], sfin[:, j : j + 1], u_ld[:],
                            op0=OP.mult, op1=OP.add,
                        )
                        if tci == NTB - 1:
                            # final state in fp32: s = U_final + P_final * s_in
                            nc.vector.scalar_tensor_tensor(
                                s_stage[:, j : j + 1],
                                p_sb[:, TT_B - 1 : TT_B],
                                sfin[:, j : j + 1],
                                carry_u[:, j : j + 1],
                                op0=OP.mult, op1=OP.add,
                            )
                    for m in range(TT_B // 128):
                        for ne in range(NE):
                            ps_y = psy_pool.tile([128, EW], f32)
                            for j in range(KB):
                                nc.tensor.matmul(
                                    ps_y[:],
                                    ys[:, j, m * 128 : (m + 1) * 128],
                                    wout[:, j, ts(ne, EW)],
                                    start=(j == 0),
                                    stop=(j == KB - 1),
                                )
                            y_st = ystage_pool.tile([128, EW], f32)
                            nc.scalar.copy(y_st[:], ps_y[:])
                            row0 = tci * TT_B + m * 128
                            nc.sync.dma_start(
                                y_d[row0 : row0 + 128, ts(ne, EW)], y_st[:]
                            )
                nc.sync.dma_start(s_d[:], s_stage[:])

    nc.compile()
    return nc


def _get_nc():
    if "nc" not in _cache:
        _cache["nc"] = _build()
    return _cache["nc"]


def kernel(x, W_in, W_out, log_A, W_dt, b_dt):
    import ml_dtypes
    from concourse.bass_utils import run_bass_kernel_spmd

    bf16 = ml_dtypes.bfloat16
    x = np.asarray(x, dtype=np.float32)
    W_in = np.asarray(W_in, dtype=np.float32)
    W_out = np.asarray(W_out, dtype=np.float32)
    log_A = np.asarray(log_A, dtype=np.float32)
    W_dt = np.asarray(W_dt, dtype=np.float32)
    b_dt = np.asarray(b_dt, dtype=np.float32)

    nc = _get_nc()

    # (j, p, k, e): W_in[j*128+e, k*128+p]
    w_in_s = np.ascontiguousarray(
        W_in.T.reshape(KB, 128, KB, 128).transpose(2, 1, 0, 3)
    ).astype(bf16)
    # (p, k, e): W_out[e, k*128+p]
    w_out_s = np.ascontiguousarray(
        W_out.T.reshape(KB, 128, D).transpose(1, 0, 2)
    ).astype(bf16)
    W_dteff = (W_dt.astype(np.float64) @ W_in.astype(np.float64)).astype(np.float32)
    w_dteff_s = np.ascontiguousarray(
        W_dteff.T.reshape(KB, 128, H).transpose(1, 0, 2)
    ).astype(bf16)
    A_flat = (-np.exp(log_A.astype(np.float64))).astype(np.float32).reshape(D)
    SA = np.zeros((H, D), dtype=np.float32)
    SA[np.arange(D) // HD, np.arange(D)] = A_flat
    bdt_col = np.ascontiguousarray(b_dt.reshape(H, 1))

    in_maps = []
    for c in range(NCORES):
        b, half = c >> 1, c & 1
        xs = x[b, half * TC : (half + 1) * TC, :]  # (TC, D)
        # (p, tc, k, tt): x[b, .. tc*TT_A+tt, k*128+p]
        xt = np.ascontiguousarray(
            xs.reshape(NTA, TT_A, KB, 128).transpose(3, 0, 2, 1)
        ).astype(bf16)
        in_maps.append(
            {
                "xt": xt,
                "w_in_s": w_in_s,
                "w_out_s": w_out_s,
                "w_dteff_s": w_dteff_s,
                "sa": SA,
                "b_dt": bdt_col,
                "cmask": np.full((128, 1), float(half), dtype=np.float32),
            }
        )

    res = run_bass_kernel_spmd(nc, in_maps, core_ids=list(range(NCORES)))
    _cache["last_res"] = res

    y = np.empty((B, T, D), dtype=np.float32)
    s = np.empty((B, H, HD), dtype=np.float32)
    for c in range(NCORES):
        r = res.results[c]
        b, half = c >> 1, c & 1
        y[b, half * TC : (half + 1) * TC, :] = r["y_out"]
        if half == 1:
            s[b] = r["s_out"].T.reshape(H, HD)
    return y, s


# revision 18
# speedup vs baseline: 1.1691x; 1.0148x over previous
"""MultiHeadSSM Trainium2 kernel (8 NeuronCores).

Module: xp = x @ W_in.T; dt = softplus(xp @ W_dt.T + b_dt);
a = exp(dt[...,None] * -exp(log_A)); linear scan s_t = a_t s_{t-1} + xp_t;
y = ys @ W_out.T; returns (y, final_state).

Sharding: 8 shards = batch(4) x T-halves(2). Core c handles b=c//2,
half=c&1 (2048 tokens, all 2048 channels). The cross-half scan dependency
is broken with the pair-scan decomposition: each core computes the local
zero-init scan U and the decay cumprod P = exp(A * cumsum(dt)); an 8KB
pair AllGather ships the first half's final state s_in, and
ys = U + P * s_in (s_in masked to 0 on even cores). dt is computed
directly from x via host-precomputed W_dteff = W_dt @ W_in, so the whole
pipeline is uniform SPMD with a single tiny collective.

On-device layout: channels on partitions, time on the free dim. Matmul
operands are bf16 (fp32 accumulation); the recurrence and all decay math
stay fp32. The recurrence runs on the DVE tensor_tensor_scan instruction.
Head->channel broadcasts are 0-stride DMA reads from DRAM, folded with
the per-channel A scale into the ACT exp.
"""
import sys

sys.path.insert(0, "/opt/trn_rl_repo")

import numpy as np

D = 2048          # d_model
H = 32            # heads
HD = 64           # head_dim
B = 4
T = 4096
NCORES = 8
TC = T // 2       # tokens per core
KB = D // 128     # 16 channel blocks
TT_A = 512        # phase-A time tile
NTA = TC // TT_A  # 4
TT_B = 512        # phase-B time tile
NTB = TC // TT_B  # 8
EW = 512          # phase-B out_proj e-chunk
NE = D // EW      # 4

_cache = {}


def _build():
    import concourse.bass as bass
    import concourse.bacc as bacc
    import concourse.tile as tile
    from concourse import mybir

    f32 = mybir.dt.float32
    bf16 = mybir.dt.bfloat16
    AF = mybir.ActivationFunctionType
    OP = mybir.AluOpType
    ts = bass.ts

    nc = bacc.Bacc("TRN2", target_bir_lowering=False, debug=False, num_devices=NCORES, num_swdge_queues=4)

    xt_d = nc.dram_tensor("xt", (128, NTA, KB, TT_A), bf16, kind="ExternalInput").ap()
    w_in_d = nc.dram_tensor("w_in_s", (KB, 128, KB, 128), bf16, kind="ExternalInput").ap()
    w_out_d = nc.dram_tensor("w_out_s", (128, KB, D), bf16, kind="ExternalInput").ap()
    w_dteff_d = nc.dram_tensor("w_dteff_s", (128, KB, H), bf16, kind="ExternalInput").ap()
    f32r = mybir.dt.float32r
    sa_d = nc.dram_tensor("sa", (H, D), f32r, kind="ExternalInput").ap()
    bdt_d = nc.dram_tensor("b_dt", (H, 1), f32, kind="ExternalInput").ap()
    cmask_d = nc.dram_tensor("cmask", (128, 1), f32, kind="ExternalInput").ap()
    y_d = nc.dram_tensor("y_out", (TC, D), f32, kind="ExternalOutput").ap()
    s_d = nc.dram_tensor("s_out", (128, KB), f32, kind="ExternalOutput").ap()

    with tile.TileContext(nc) as tc:
        with (
            tc.tile_pool(name="resident", bufs=1) as rpool,
            tc.tile_pool(name="cpool", bufs=1) as cpool,
            tc.tile_pool(name="dram", bufs=1, space="DRAM") as dram,
        ):
            u_dram = dram.tile([D, TC], bf16)
            p_dram = dram.tile([D, TC], bf16)
            cc_in1 = dram.tile([12, 128], f32)
            cc_out1 = dram.tile([2, 12, 128], f32)
            cc_in2 = dram.tile([4, 128], f32)
            cc_out2 = dram.tile([2, 4, 128], f32)
            u_r = u_dram[:].rearrange("(j p) t -> p j t", p=128)
            p_r = p_dram[:].rearrange("(j p) t -> p j t", p=128)

            # resident big tensors: x (bf16) and W_out (bf16) together
            xsb = rpool.tile([128, NTA, KB, TT_A], bf16)
            wout = rpool.tile([128, KB, D], bf16)
            for k in range(KB):
                nc.sync.dma_start(xsb[:, 0, k, :], xt_d[:, 0, k, :])
            for tci in range(1, NTA):
                for kq in range(4):
                    nc.sync.dma_start(
                        xsb[:, tci, 4 * kq : 4 * kq + 4, :],
                        xt_d[:, tci, 4 * kq : 4 * kq + 4, :],
                    )

            wdteff = cpool.tile([128, KB, H], bf16)
            sa_sb = cpool.tile([H, D], f32r)
            dt_f32r = cpool.tile([H, TC], f32r)
            bdt_sb = cpool.tile([H, 1], f32)
            cmask_sb = cpool.tile([128, 1], f32)
            carry_u = cpool.tile([128, KB], f32)
            sfin_raw = cpool.tile([128, KB], f32)
            sfin = cpool.tile([128, KB], f32)
            s_stage = cpool.tile([128, KB], f32)
            carry_p = cpool.tile([128, KB], f32)
            zeros32 = cpool.tile([H, TT_A], f32)
            zeros128 = cpool.tile([128, TT_A], f32)
            carry_s32 = cpool.tile([H, 1], f32)
            nc.gpsimd.memset(zeros32[:], 0.0)
            nc.gpsimd.memset(zeros128[:], 0.0)
            nc.sync.dma_start(wdteff[:], w_dteff_d[:])
            nc.sync.dma_start(sa_sb[:], sa_d[:])
            nc.sync.dma_start(bdt_sb[:], bdt_d[:])
            nc.sync.dma_start(cmask_sb[:], cmask_d[:])

            # ---------------- phase A: dt, cumsum(dt), in_proj, scan U ------
            with (
                tc.tile_pool(name="psdt", bufs=1, space=bass.MemorySpace.PSUM) as psdt_pool,
                tc.tile_pool(name="psxp", bufs=5, space=bass.MemorySpace.PSUM) as psxp_pool,
                tc.tile_pool(name="psdtb", bufs=2, space=bass.MemorySpace.PSUM) as psdtb_pool,
                tc.tile_pool(name="win", bufs=3) as win_pool,
                tc.tile_pool(name="apool", bufs=5) as a_pool,
                tc.tile_pool(name="upool", bufs=4) as u_pool,
                tc.tile_pool(name="pspool", bufs=4) as pp_pool,
                tc.tile_pool(name="spool", bufs=2) as small_pool,
            ):
                wj_tiles = {}
                for j in range(2):
                    wj_tiles[j] = win_pool.tile([128, KB, 128], bf16, tag="wj", name=f"wj{j}")
                    nc.gpsimd.dma_start(wj_tiles[j][:], w_in_d[j])

                # dt = softplus(W_dteff @ x + b)
                for tci in range(NTA):
                    ps_dt = psdt_pool.tile([H, TT_A], f32)
                    for k in range(KB):
                        nc.tensor.matmul(
                            ps_dt[:],
                            wdteff[:, k, :],
                            xsb[:, tci, k, :],
                            start=(k == 0),
                            stop=(k == KB - 1),
                        )
                    e_sb = small_pool.tile([H, TT_A], f32, tag="esb")
                    nc.scalar.activation(e_sb[:], ps_dt[:], AF.Exp, bias=bdt_sb[:, 0:1])
                    nc.scalar.activation(
                        dt_f32r[:, ts(tci, TT_A)], e_sb[:], AF.Ln, bias=1.0
                    )

                for k in range(KB):
                    nc.sync.dma_start(wout[:, k, :], w_out_d[:, k, :])

                for j in range(KB):
                    wj = wj_tiles.pop(j)
                    if j + 2 < KB:
                        wj_tiles[j + 2] = win_pool.tile([128, KB, 128], bf16, tag="wj", name=f"wj{j+2}")
                        nc.gpsimd.dma_start(wj_tiles[j + 2][:], w_in_d[j + 2])
                    for tci in range(NTA):
                        ps_xp = psxp_pool.tile([128, TT_A], f32)
                        for k in range(KB):
                            nc.tensor.matmul(
                                ps_xp[:],
                                wj[:, k, :],
                                xsb[:, tci, k, :],
                                start=(k == 0),
                                stop=(k == KB - 1),
                            )
                        ps_dtb = psdtb_pool.tile([128, TT_A], f32)
                        nc.tensor.matmul(
                            ps_dtb[:],
                            sa_sb[:, ts(j, 128)],
                            dt_f32r[:, ts(tci, TT_A)],
                            start=True,
                            stop=True,
                        )
                        a_sb = a_pool.tile([128, TT_A], f32)
                        nc.scalar.activation(a_sb[:], ps_dtb[:], AF.Exp)
                        u_sb = u_pool.tile([128, TT_A], bf16)
                        init = 0.0 if tci == 0 else carry_u[:, j : j + 1]
                        nc.vector.tensor_tensor_scan(
                            u_sb[:], a_sb[:], ps_xp[:], init,
                            op0=OP.mult, op1=OP.add,
                        )
                        nc.vector.tensor_copy(
                            carry_u[:, j : j + 1], u_sb[:, TT_A - 1 : TT_A]
                        )
                        nc.sync.dma_start(u_r[:, j, ts(tci, TT_A)], u_sb[:])
                        p_sb = pp_pool.tile([128, TT_A], bf16)
                        initp = 1.0 if tci == 0 else carry_p[:, j : j + 1]
                        nc.vector.tensor_tensor_scan(
                            p_sb[:], a_sb[:], zeros128[:], initp,
                            op0=OP.mult, op1=OP.add,
                        )
                        nc.vector.tensor_copy(
                            carry_p[:, j : j + 1], p_sb[:, TT_A - 1 : TT_A]
                        )
                        nc.sync.dma_start(p_r[:, j, ts(tci, TT_A)], p_sb[:])
                    if j == 11:
                        nc.sync.dma_start(
                            cc_in1[:].rearrange("j p -> p j"), carry_u[:, 0:12]
                        )
                        nc.gpsimd.collective_compute(
                            "AllGather",
                            OP.bypass,
                            replica_groups=[[0, 1], [2, 3], [4, 5], [6, 7]],
                            ins=[cc_in1.opt()],
                            outs=[cc_out1.opt()],
                        )

            # ---------------- pair handoff of final local states ------------
            nc.sync.dma_start(cc_in2[:].rearrange("j p -> p j"), carry_u[:, 12:16])
            nc.gpsimd.collective_compute(
                "AllGather",
                OP.bypass,
                replica_groups=[[0, 1], [2, 3], [4, 5], [6, 7]],
                ins=[cc_in2.opt()],
                outs=[cc_out2.opt()],
            )
            nc.sync.dma_start(sfin_raw[:, 0:12], cc_out1[0].rearrange("j p -> p j"))
            nc.sync.dma_start(sfin_raw[:, 12:16], cc_out2[0].rearrange("j p -> p j"))
            nc.vector.tensor_scalar_mul(
                sfin[:, 0:12], sfin_raw[:, 0:12], cmask_sb[:, 0:1]
            )
            nc.vector.tensor_scalar_mul(
                sfin[:, 12:16], sfin_raw[:, 12:16], cmask_sb[:, 0:1]
            )

            # ---------------- phase B: P, correction, out_proj --------------
            with (
                tc.tile_pool(name="psy", bufs=3, space=bass.MemorySpace.PSUM) as psy_pool,
                tc.tile_pool(name="uld", bufs=8) as uld_pool,
                tc.tile_pool(name="pld", bufs=8) as pld_pool,
                tc.tile_pool(name="yspool", bufs=2) as ys_pool,
                tc.tile_pool(name="ystage", bufs=3) as ystage_pool,
            ):
                for tci in range(NTB):
                    ys = ys_pool.tile([128, KB, TT_B], bf16, tag="ys")
                    for j in range(KB):
                        u_ld = uld_pool.tile([128, TT_B], bf16)
                        nc.sync.dma_start(u_ld[:], u_r[:, j, ts(tci, TT_B)])
                        p_sb = pld_pool.tile([128, TT_B], bf16)
                        nc.gpsimd.dma_start(p_sb[:], p_r[:, j, ts(tci, TT_B)])
                        nc.vector.scalar_tensor_tensor(
                            ys[:, j, :], p_sb[:], sfin[:, j : j + 1], u_ld[:],
                            op0=OP.mult, op1=OP.add,
                        )
                        if tci == NTB - 1:
                            # final state in fp32: s = U_final + P_final * s_in
                            nc.vector.scalar_tensor_tensor(
                                s_stage[:, j : j + 1],
                                p_sb[:, TT_B - 1 : TT_B],
                                sfin[:, j : j + 1],
                                carry_u[:, j : j + 1],
                                op0=OP.mult, op1=OP.add,
                            )
                    for m in range(TT_B // 128):
                        for ne in range(NE):
                            ps_y = psy_pool.tile([128, EW], f32)
                            for j in range(KB):
                                nc.tensor.matmul(
                                    ps_y[:],
                                    ys[:, j, m * 128 : (m + 1) * 128],
                                    wout[:, j, ts(ne, EW)],
                                    start=(j == 0),
                                    stop=(j == KB - 1),
                                )
                            y_st = ystage_pool.tile([128, EW], f32)
                            nc.scalar.copy(y_st[:], ps_y[:])
                            row0 = tci * TT_B + m * 128
                            nc.sync.dma_start(
                                y_d[row0 : row0 + 128, ts(ne, EW)], y_st[:]
                            )
                nc.sync.dma_start(s_d[:], s_stage[:])

    nc.compile()
    return nc


def _get_nc():
    if "nc" not in _cache:
        _cache["nc"] = _build()
    return _cache["nc"]


def kernel(x, W_in, W_out, log_A, W_dt, b_dt):
    import ml_dtypes
    from concourse.bass_utils import run_bass_kernel_spmd

    bf16 = ml_dtypes.bfloat16
    x = np.asarray(x, dtype=np.float32)
    W_in = np.asarray(W_in, dtype=np.float32)
    W_out = np.asarray(W_out, dtype=np.float32)
    log_A = np.asarray(log_A, dtype=np.float32)
    W_dt = np.asarray(W_dt, dtype=np.float32)
    b_dt = np.asarray(b_dt, dtype=np.float32)

    nc = _get_nc()

    # (j, p, k, e): W_in[j*128+e, k*128+p]
    w_in_s = np.ascontiguousarray(
        W_in.T.reshape(KB, 128, KB, 128).transpose(2, 1, 0, 3)
    ).astype(bf16)
    # (p, k, e): W_out[e, k*128+p]
    w_out_s = np.ascontiguousarray(
        W_out.T.reshape(KB, 128, D).transpose(1, 0, 2)
    ).astype(bf16)
    W_dteff = (W_dt.astype(np.float64) @ W_in.astype(np.float64)).astype(np.float32)
    w_dteff_s = np.ascontiguousarray(
        W_dteff.T.reshape(KB, 128, H).transpose(1, 0, 2)
    ).astype(bf16)
    A_flat = (-np.exp(log_A.astype(np.float64))).astype(np.float32).reshape(D)
    SA = np.zeros((H, D), dtype=np.float32)
    SA[np.arange(D) // HD, np.arange(D)] = A_flat
    bdt_col = np.ascontiguousarray(b_dt.reshape(H, 1))

    in_maps = []
    for c in range(NCORES):
        b, half = c >> 1, c & 1
        xs = x[b, half * TC : (half + 1) * TC, :]  # (TC, D)
        # (p, tc, k, tt): x[b, .. tc*TT_A+tt, k*128+p]
        xt = np.ascontiguousarray(
            xs.reshape(NTA, TT_A, KB, 128).transpose(3, 0, 2, 1)
        ).astype(bf16)
        in_maps.append(
            {
                "xt": xt,
                "w_in_s": w_in_s,
                "w_out_s": w_out_s,
                "w_dteff_s": w_dteff_s,
                "sa": SA,
                "b_dt": bdt_col,
                "cmask": np.full((128, 1), float(half), dtype=np.float32),
            }
        )

    res = run_bass_kernel_spmd(nc, in_maps, core_ids=list(range(NCORES)))
    _cache["last_res"] = res

    y = np.empty((B, T, D), dtype=np.float32)
    s = np.empty((B, H, HD), dtype=np.float32)
    for c in range(NCORES):
        r = res.results[c]
        b, half = c >> 1, c & 1
        y[b, half * TC : (half + 1) * TC, :] = r["y_out"]
        if half == 1:
            s[b] = r["s_out"].T.reshape(H, HD)
    return y, s
